# revision 11
# baseline (speedup 1.0000x reference)
"""Trainium2 Bass kernel for nn_CDC_62646392980082 (GRU-CPC loss_fn).

Contract: kernel(**inputs) takes the FULL unsharded inputs (numpy) and
returns the FULL output (loss, acc) exactly like the jax reference.

Strategy (8 NeuronCores, data-parallel over batch B=256 -> 32/core):
  - Transposed layouts (feature dims on SBUF partitions) so every
    contraction is a clean PE matmul; bf16 matmuls with fp32 PSUM
    accumulate and fp32 gate/softmax arithmetic.
  - Host pre-transposes weights/encodings once so all DMAs are
    contiguous; negatives are folded host-side into per-(prediction,
    cell) multiplicity counts so the random gather becomes dense masked
    reductions on the DVE.
  - Per-core partial sums of (loss, correct) are summed on host.
"""

import sys

if "/opt/trn_rl_repo" not in sys.path:
    sys.path.insert(0, "/opt/trn_rl_repo")

import numpy as np
import ml_dtypes

B, K, R, C, P, H, S = 256, 5, 6, 7, 1280, 256, 64
NCORE = 8
BS = B // NCORE            # 32 images per core
BC = BS * C                # 224 (b, c) columns
PC_N = P // 128            # 10 p-chunks
HC_N = H // 128            # 2 h-chunks
IJ = 49                    # 7x7 cells
PAIRS = [(k, r) for k in range(K) for r in range(R - k)]   # 20 valid (k, r)
NPAIR = len(PAIRS)
HALF = 10                  # pairs per pass
N_PREDS = NPAIR * B * C    # 35840 global predictions

_CACHE = {}


def _build_program():
    import concourse.bacc as bacc
    import concourse.mybir as mybir
    from concourse.tile import TileContext

    f32 = mybir.dt.float32
    bf16 = mybir.dt.float16  # fp16: same PE rate as bf16, 4x mantissa
    Alu = mybir.AluOpType
    Act = mybir.ActivationFunctionType

    nc = bacc.Bacc()
    dp = nc.declare_dram_parameter
    encT = dp("encT", [PC_N, 128, R * BC], bf16, isOutput=False)   # GRU layout
    encB = dp("encB", [PC_N, 128, BS * IJ], bf16, isOutput=False)  # dots layout
    wih = dp("wih", [PC_N, 128, 768], bf16, isOutput=False)
    whh = dp("whh", [HC_N, 128, 768], bf16, isOutput=False)
    wk = dp("wk", [K, HC_N, 128, P], bf16, isOutput=False)
    brz = dp("brz", [128, 4], f32, isOutput=False)
    bihn = dp("bihn", [128, 2], f32, isOutput=False)
    bhhn = dp("bhhn", [128, 2], f32, isOutput=False)
    wkb = dp("wkb", [128, K * PC_N], f32, isOutput=False)
    cnt1 = dp("cnt1", [70, 2 * BS * IJ], bf16, isOutput=False)
    posm = dp("posm", [70, 2 * IJ], f32, isOutput=False)
    out = dp("out", [1, 2], f32, isOutput=True)

    with TileContext(nc, pool_alloc_mode="queue") as tc:
        with (
            tc.tile_pool(name="pers", bufs=1) as pers,
            tc.tile_pool(name="psGH", bufs=3, space="PSUM") as psGH,
        ):
            # ---- persistent small loads ----
            brz_t = pers.tile([128, 4], f32)
            nc.sync.dma_start(out=brz_t, in_=brz[:, :])
            bihn_t = pers.tile([128, 2], f32)
            nc.sync.dma_start(out=bihn_t, in_=bihn[:, :])
            bhhn_t = pers.tile([128, 2], f32)
            nc.sync.dma_start(out=bhhn_t, in_=bhhn[:, :])
            wkb_t = pers.tile([128, K * PC_N], f32)
            nc.sync.dma_start(out=wkb_t, in_=wkb[:, :])
            whh_t = [pers.tile([128, 768], bf16, tag=f"whh{h}", name=f"whh{h}") for h in range(HC_N)]
            for h in range(HC_N):
                nc.sync.dma_start(out=whh_t[h], in_=whh[h, :, :])

            # zero initial hidden state (bf16)
            zb = pers.tile([128, 256], bf16)
            nc.vector.memset(zb, 0.0)

            # GRU context: per-(h-chunk, r-pair) tiles [128, 512] bf16;
            # each r block is 256 cols = 224 real + 32 pad (zeroed)
            ctxp = [
                [pers.tile([128, 512], bf16, tag=f"ctx{h}_{rp}", name=f"ctx{h}_{rp}") for rp in range(R // 2)]
                for h in range(HC_N)
            ]
            for h in range(HC_N):
                for rp in range(R // 2):
                    pv = ctxp[h][rp].rearrange("p (q x) -> p q x", q=2)[:, :, BC:]
                    nc.vector.memset(pv, 0.0)

            def ctx_r(h, r):
                return ctxp[h][r // 2][:, (r % 2) * 256 : (r % 2) * 256 + 256]

            outS = pers.tile([1, 2], f32)
            # gi chunks of 3 steps each (672 cols)
            GI_CH = [(0, 672), (672, 672)]
            gis = [
                [pers.tile([128, w], f32, tag=f"gis{m}_{c}", name=f"gis{m}_{c}") for c, (o, w) in enumerate(GI_CH)]
                for m in range(6)
            ]

            def gi_slice(m, r):
                ci, rem = divmod(r, 3)
                return gis[m][ci][:, rem * BC : (rem + 1) * BC]

            # ---- phase 1: gi = x @ W_ih.T, interleaved with GRU steps ----
            with (
                tc.tile_pool(name="p1", bufs=1) as p1,
                tc.tile_pool(name="psGI", bufs=2, space="PSUM") as psGI,
            ):
                enc_t = [p1.tile([128, R * BC], bf16, tag=f"enc{i}", name=f"enc{i}") for i in range(PC_N)]
                wih_t = [p1.tile([128, 768], bf16, tag=f"wih{i}", name=f"wih{i}") for i in range(PC_N)]
                for i in range(PC_N):
                    nc.sync.dma_start(out=enc_t[i], in_=encT[i, :, :])
                    nc.sync.dma_start(out=wih_t[i], in_=wih[i, :, :])

                def emit_gi_chunk(ci):
                    off, w = GI_CH[ci]
                    for m in range(6):
                        ps = psGI.tile([128, 512], f32, tag="gi", name=f"gi_{ci}_{m}")
                        for h2 in range(2):          # 672 = 336+336 (<=512 psum)
                            lo, wd = h2 * 336, 336
                            for pc in range(PC_N):
                                nc.tensor.matmul(
                                    ps[:, :wd],
                                    wih_t[pc][:, m * 128 : (m + 1) * 128],
                                    enc_t[pc][:, off + lo : off + lo + wd],
                                    start=(pc == 0),
                                    stop=(pc == PC_N - 1),
                                )
                            nc.vector.tensor_copy(
                                gis[m][ci][:, lo : lo + wd], ps[:, :wd]
                            )

                def emit_gru_step(r):
                    hprev = [zb, zb] if r == 0 else [ctx_r(h, r - 1) for h in range(HC_N)]
                    ghp = []
                    for m in range(6):
                        ps = psGH.tile([128, 256], f32, tag="gh", name=f"gh_{r}_{m}")
                        for hc in range(HC_N):
                            nc.tensor.matmul(
                                ps,
                                whh_t[hc][:, m * 128 : (m + 1) * 128],
                                hprev[hc],
                                start=(hc == 0),
                                stop=(hc == HC_N - 1),
                            )
                        ghp.append(ps)
                    for t in range(2):
                        iR = gi_slice(0 + t, r)
                        iZ = gi_slice(2 + t, r)
                        iN = gi_slice(4 + t, r)
                        hR = ghp[0 + t][:, :BC]
                        hZ = ghp[2 + t][:, :BC]
                        hN = ghp[4 + t][:, :BC]
                        tA = pers.tile([128, BC], f32, tag="tA", bufs=2, name=f"tA{r}{t}")
                        nc.vector.tensor_tensor(tA, iR, hR, op=Alu.add)
                        rt = pers.tile([128, BC], f32, tag="rt", bufs=2, name=f"rt{r}{t}")
                        nc.scalar.activation(rt, tA, Act.Sigmoid, bias=brz_t[:, 0 + t : 1 + t])
                        tB = pers.tile([128, BC], f32, tag="tB", bufs=2, name=f"tB{r}{t}")
                        nc.vector.tensor_tensor(tB, iZ, hZ, op=Alu.add)
                        zt = pers.tile([128, BC], f32, tag="zt", bufs=2, name=f"zt{r}{t}")
                        nc.scalar.activation(zt, tB, Act.Sigmoid, bias=brz_t[:, 2 + t : 3 + t])
                        tV = pers.tile([128, BC], f32, tag="tV", bufs=2, name=f"tV{r}{t}")
                        nc.vector.scalar_tensor_tensor(
                            tV, hN, bhhn_t[:, t : t + 1], rt, op0=Alu.add, op1=Alu.mult
                        )
                        tW = pers.tile([128, BC], f32, tag="tW", bufs=2, name=f"tW{r}{t}")
                        nc.vector.tensor_tensor(tW, tV, iN, op=Alu.add)
                        nt = pers.tile([128, BC], f32, tag="nt", bufs=2, name=f"nt{r}{t}")
                        nc.scalar.activation(nt, tW, Act.Tanh, bias=bihn_t[:, t : t + 1])
                        tD = pers.tile([128, BC], f32, tag="tD", bufs=2, name=f"tD{r}{t}")
                        nc.vector.tensor_tensor(tD, hprev[t][:, :BC], nt, op=Alu.subtract)
                        tE = pers.tile([128, BC], f32, tag="tE", bufs=2, name=f"tE{r}{t}")
                        nc.vector.tensor_tensor(tE, zt, tD, op=Alu.mult)
                        hout = ctx_r(t, r)[:, :BC]
                        nc.vector.tensor_tensor(hout, nt, tE, op=Alu.add)

                # interleave emission so GRU overlaps the gi tail
                emit_gi_chunk(0)
                emit_gru_step(0)
                emit_gi_chunk(1)
                for r in range(1, R):
                    emit_gru_step(r)

            # ---- phase 3: preds (clip(ctx @ Wk.T + b)) + dots + loss ----
            with (
                tc.tile_pool(name="pp", bufs=1) as ppool,
                tc.tile_pool(name="psPP", bufs=2, space="PSUM") as psPP,
                tc.tile_pool(name="psDP", bufs=2, space="PSUM") as psDP,
            ):
                predsT = [
                    ppool.tile([128, BS * HALF * C], bf16, tag=f"pt{i}", name=f"pt{i}")
                    for i in range(PC_N)
                ]
                encB_t = [
                    ppool.tile([128, BS * IJ], bf16, tag=f"eb{i}", name=f"eb{i}")
                    for i in range(PC_N)
                ]
                for i in range(PC_N):
                    nc.sync.dma_start(out=encB_t[i], in_=encB[i, :, :])
                posm_t = ppool.tile([70, 2 * IJ], f32)
                nc.sync.dma_start(out=posm_t, in_=posm[:, :])
                cnt1_t = ppool.tile([70, 2 * BS * IJ], bf16)
                nc.sync.dma_start(out=cnt1_t, in_=cnt1[:, :])
                D = ppool.tile([70, 2 * BS * IJ], f32)
                B2 = ppool.tile([70, 2 * BS * IJ], f32)
                G2 = BS  # groups per half
                mx = ppool.tile([70, 2 * G2], f32, tag="mx")
                se = ppool.tile([70, 2 * G2], f32, tag="se")
                pos = ppool.tile([70, 2 * G2], f32, tag="pos")
                lnv = ppool.tile([70, 2 * G2], f32, tag="lnv")
                corr = ppool.tile([70, 2 * G2], f32, tag="corr")
                Ssum = ppool.tile([70, 4], f32, tag="S")

                def emit_preds_pass(pass_i):
                    ppairs = PAIRS[pass_i * HALF : (pass_i + 1) * HALF]
                    runs = []
                    q = 0
                    while q < HALF:
                        k = ppairs[q][0]
                        q0 = q
                        while q < HALF and ppairs[q][0] == k:
                            q += 1
                        runs.append((k, q0, q))
                    for k, q0, q1 in runs:
                        wk_t = []
                        for hc in range(HC_N):
                            w = ppool.tile(
                                [128, P], bf16, tag=f"wk{hc}", bufs=2,
                                name=f"wk{pass_i}_{k}_{hc}",
                            )
                            nc.sync.dma_start(out=w, in_=wk[k, hc, :, :])
                            wk_t.append(w)
                        for qc in range(q0, q1, 2):
                            nq = min(2, q1 - qc)        # 2 -> N=512, 1 -> N=256
                            rs = [ppairs[qc + i][1] for i in range(nq)]
                            for m in range(PC_N):
                                ps = psPP.tile(
                                    [128, 512], f32, tag="pp", name=f"pp_{pass_i}_{qc}_{m}"
                                )
                                for hc in range(HC_N):
                                    if nq == 2:
                                        assert rs[1] == rs[0] + 1 and rs[0] % 2 == 0
                                        rhs = ctxp[hc][rs[0] // 2]
                                    else:
                                        rhs = ctx_r(hc, rs[0])
                                    nc.tensor.matmul(
                                        ps[:, : nq * 256],
                                        wk_t[hc][:, m * 128 : (m + 1) * 128],
                                        rhs,
                                        start=(hc == 0),
                                        stop=(hc == HC_N - 1),
                                    )
                                at = ppool.tile([128, nq * BC], f32, tag="at", bufs=3,
                                                name=f"at_{pass_i}_{qc}_{m}")
                                psv = ps.rearrange("p (q x) -> p q x", q=2)[:, :nq, :BC]
                                atv = at.rearrange("p (q x) -> p q x", q=nq)
                                nc.vector.tensor_scalar(
                                    atv, psv,
                                    wkb_t[:, k * PC_N + m : k * PC_N + m + 1],
                                    -1.0, Alu.add, Alu.max,
                                )
                                dst = predsT[m].rearrange(
                                    "p (b q c) -> p q b c", b=BS, q=HALF
                                )[:, qc : qc + nq, :, :]
                                src = at.rearrange("p (q b c) -> p q b c", q=nq, b=BS)
                                nc.gpsimd.tensor_scalar(
                                    dst, src, 1.0, None, Alu.min
                                )

                def emit_dots_pass(pass_i):
                    for b in range(BS):
                        ps = psDP.tile([70, IJ], f32, tag="dp", name=f"dp{pass_i}_{b}")
                        for pc in range(PC_N):
                            nc.tensor.matmul(
                                ps,
                                predsT[pc][:, b * 70 : (b + 1) * 70],
                                encB_t[pc][:, b * IJ : (b + 1) * IJ],
                                start=(pc == 0),
                                stop=(pc == PC_N - 1),
                            )
                        nc.vector.tensor_copy(
                            D[:, (pass_i * BS + b) * IJ : (pass_i * BS + b + 1) * IJ],
                            ps,
                        )

                def emit_post_half(h):
                    # groups of this half: cols [h*G2*IJ, (h+1)*G2*IJ)
                    lo = h * G2 * IJ
                    hi = (h + 1) * G2 * IJ
                    Dh = D[:, lo:hi]
                    B2h = B2[:, lo:hi]
                    Dv = Dh.rearrange("p (g j) -> p g j", j=IJ)
                    B2v = B2h.rearrange("p (g j) -> p g j", j=IJ)
                    cnt_h = cnt1_t[:, lo:hi]
                    gsl = slice(h * G2, (h + 1) * G2)
                    mxh = mx[:, gsl]
                    seh = se[:, gsl]
                    posh = pos[:, gsl]
                    lnvh = lnv[:, gsl]
                    corrh = corr[:, gsl]
                    nc.vector.tensor_scalar(B2h, cnt_h, 0.0, -1e30, Alu.is_equal, Alu.mult)
                    nc.vector.tensor_tensor(Dh, Dh, B2h, op=Alu.add)
                    nc.vector.tensor_reduce(mxh, Dv, axis=mybir.AxisListType.X, op=Alu.max)
                    nc.vector.tensor_tensor(
                        B2v, Dv, mxh.unsqueeze(2).broadcast_to([70, G2, IJ]), op=Alu.subtract
                    )
                    nc.scalar.activation(B2h, B2h, Act.Exp)
                    nc.vector.tensor_tensor(B2h, B2h, cnt_h, op=Alu.mult)
                    nc.vector.tensor_reduce(seh, B2v, axis=mybir.AxisListType.X, op=Alu.add)
                    # pos = sum(D * posmask) (exact: zeros elsewhere)
                    pmh = posm_t[:, h * IJ : (h + 1) * IJ]
                    nc.vector.tensor_tensor(
                        B2v, Dv, pmh.unsqueeze(1).broadcast_to([70, G2, IJ]), op=Alu.mult
                    )
                    nc.vector.tensor_reduce(posh, B2v, axis=mybir.AxisListType.X, op=Alu.add)
                    # loss = ln(se) + mx - pos ; correct = (pos >= mx)
                    nc.scalar.activation(lnvh, seh, Act.Ln)
                    nc.vector.tensor_tensor(lnvh, lnvh, mxh, op=Alu.add)
                    nc.vector.tensor_tensor(corrh, posh, mxh, op=Alu.is_ge)
                    nc.vector.tensor_tensor(lnvh, lnvh, posh, op=Alu.subtract)
                    nc.vector.tensor_reduce(
                        Ssum[:, 2 * h : 2 * h + 1], lnvh,
                        axis=mybir.AxisListType.X, op=Alu.add,
                    )
                    nc.vector.tensor_reduce(
                        Ssum[:, 2 * h + 1 : 2 * h + 2], corrh,
                        axis=mybir.AxisListType.X, op=Alu.add,
                    )

                emit_preds_pass(0)
                emit_dots_pass(0)
                emit_preds_pass(1)
                emit_post_half(0)
                emit_dots_pass(1)
                emit_post_half(1)

                # combine halves: [loss, acc] = colsums of Ssum (half0|half1)
                ones = ppool.tile([70, 1], f32, tag="ones")
                nc.vector.memset(ones, 1.0)
                fp = psDP.tile([1, 4], f32, tag="dp", name="fin")
                nc.tensor.matmul(fp, ones, Ssum, start=True, stop=True)
                fs = ppool.tile([1, 4], f32, tag="fs")
                nc.vector.tensor_copy(fs, fp)
                nc.vector.tensor_tensor(outS, fs[:, 0:2], fs[:, 2:4], op=Alu.add)
                nc.sync.dma_start(out=out[:, :], in_=outS)

    nc.finalize()
    return nc


def _prep_inputs(encodings, hidden, W_ih, W_hh, b_ih, b_hh, Wk_w, Wk_b,
                 neg_rows, neg_cols):
    """Host-side reformat of the full inputs into per-core DMA-clean arrays."""
    bf16 = np.float16
    enc = np.ascontiguousarray(encodings, dtype=np.float32)
    e6 = enc.reshape(NCORE, BS, C, C, PC_N, 128)  # (core, b, i, c, pc, pp)
    # GRU layout: [core, pc, pp, r*BC + b*7 + c], r < 6
    encT = np.ascontiguousarray(
        e6[:, :, :R].transpose(0, 4, 5, 2, 1, 3)
    ).reshape(NCORE, PC_N, 128, R * BC).astype(bf16)
    # dots layout: [core, pc, pp, b*49 + i*7 + c]
    encB = np.ascontiguousarray(
        e6.transpose(0, 4, 5, 1, 2, 3)
    ).reshape(NCORE, PC_N, 128, BS * IJ).astype(bf16)

    wih = np.ascontiguousarray(W_ih.T, dtype=np.float32).reshape(PC_N, 128, 768).astype(bf16)
    whh = np.ascontiguousarray(W_hh.T, dtype=np.float32).reshape(HC_N, 128, 768).astype(bf16)
    wkh = np.ascontiguousarray(
        Wk_w.transpose(0, 2, 1), dtype=np.float32
    ).reshape(K, HC_N, 128, P).astype(bf16)
    bsum = (b_ih + b_hh).astype(np.float32)
    brz = np.ascontiguousarray(bsum[:512].reshape(4, 128).T)
    bihn = np.ascontiguousarray(b_ih[512:].astype(np.float32).reshape(2, 128).T)
    bhhn = np.ascontiguousarray(b_hh[512:].astype(np.float32).reshape(2, 128).T)
    wkb = np.ascontiguousarray(
        Wk_b.astype(np.float32).reshape(K, PC_N, 128).transpose(2, 0, 1)
    ).reshape(128, K * PC_N)

    # negatives -> multiplicity counts over the 49 cells, plus the positive
    neg_idx = (neg_rows.astype(np.int64) * 7 + neg_cols.astype(np.int64))  # [B,K,R,C,63]
    sel = np.stack([neg_idx[:, k, r] for (k, r) in PAIRS], axis=1)  # [B,20,C,63]
    flat = (
        np.arange(B * NPAIR * C, dtype=np.int64)[:, None] * IJ
        + sel.reshape(B * NPAIR * C, S - 1)
    ).ravel()
    cnts = np.bincount(flat, minlength=B * NPAIR * C * IJ).reshape(
        B, NPAIR, C, IJ
    ).astype(np.float32)
    cvec = np.arange(C)
    for pi, (k, r) in enumerate(PAIRS):
        cnts[:, pi, cvec, r * 7 + cvec] += 1.0   # include the positive
    # device layout [core, row=q*7+c, half, b_local, j]
    cnt1 = np.ascontiguousarray(
        cnts.reshape(NCORE, BS, 2, HALF, C, IJ).transpose(0, 3, 4, 2, 1, 5)
    ).reshape(NCORE, HALF * C, 2 * BS * IJ).astype(bf16)

    posm = np.zeros((HALF * C, 2, IJ), dtype=np.float32)
    for half in range(2):
        for qq in range(HALF):
            k, r = PAIRS[half * HALF + qq]
            for c in range(C):
                posm[qq * 7 + c, half, r * 7 + c] = 1.0
    posm = posm.reshape(HALF * C, 2 * IJ)

    in_maps = []
    for core in range(NCORE):
        in_maps.append(
            {
                "encT": encT[core],
                "encB": encB[core],
                "wih": wih,
                "whh": whh,
                "wk": wkh,
                "brz": brz,
                "bihn": bihn,
                "bhhn": bhhn,
                "wkb": wkb,
                "cnt1": cnt1[core],
                "posm": posm,
            }
        )
    return in_maps


def _get_program():
    if "nc" not in _CACHE:
        _CACHE["nc"] = _build_program()
    return _CACHE["nc"]


def run_on_device(in_maps, trace=False, tmpdir=None):
    from concourse.bass_utils import run_bass_kernel_spmd

    nc = _get_program()
    return run_bass_kernel_spmd(
        nc, in_maps, list(range(NCORE)), trace=trace, tmpdir=tmpdir
    )


def kernel(**inputs):
    in_maps = _prep_inputs(**inputs)
    res = run_on_device(in_maps)
    loss_sum = 0.0
    corr_sum = 0.0
    for core in range(NCORE):
        o = res.results[core]["out"]
        loss_sum += float(o[0, 0])
        corr_sum += float(o[0, 1])
    loss = np.float32(loss_sum / N_PREDS)
    acc = np.float32(corr_sum / N_PREDS)
    return loss, acc


# revision 13
# speedup vs baseline: 3.4853x; 3.4853x over previous
"""Trainium2 Bass kernel for nn_CDC_62646392980082 (GRU-CPC loss_fn).

Contract: kernel(**inputs) takes the FULL unsharded inputs (numpy) and
returns the FULL output (loss, acc) exactly like the jax reference.

Strategy (8 NeuronCores, data-parallel over batch B=256 -> 32/core):
  - Transposed layouts (feature dims on SBUF partitions) so every
    contraction is a clean PE matmul; bf16 matmuls with fp32 PSUM
    accumulate and fp32 gate/softmax arithmetic.
  - Host pre-transposes weights/encodings once so all DMAs are
    contiguous; negatives are folded host-side into per-(prediction,
    cell) multiplicity counts so the random gather becomes dense masked
    reductions on the DVE.
  - Per-core partial sums of (loss, correct) are summed on host.
"""

import sys

if "/opt/trn_rl_repo" not in sys.path:
    sys.path.insert(0, "/opt/trn_rl_repo")

import numpy as np
import ml_dtypes

B, K, R, C, P, H, S = 256, 5, 6, 7, 1280, 256, 64
NCORE = 8
BS = B // NCORE            # 32 images per core
BC = BS * C                # 224 (b, c) columns
PC_N = P // 128            # 10 p-chunks
HC_N = H // 128            # 2 h-chunks
IJ = 49                    # 7x7 cells
PAIRS = [(k, r) for k in range(K) for r in range(R - k)]   # 20 valid (k, r)
NPAIR = len(PAIRS)
HALF = 10                  # pairs per pass
N_PREDS = NPAIR * B * C    # 35840 global predictions

_CACHE = {}


def _build_program():
    import concourse.bacc as bacc
    import concourse.mybir as mybir
    from concourse.tile import TileContext

    f32 = mybir.dt.float32
    bf16 = mybir.dt.float16  # fp16: same PE rate as bf16, 4x mantissa
    Alu = mybir.AluOpType
    Act = mybir.ActivationFunctionType

    nc = bacc.Bacc()
    dp = nc.declare_dram_parameter
    encT = dp("encT", [PC_N, 128, R * BC], bf16, isOutput=False)   # GRU layout
    encB = dp("encB", [PC_N, 128, BS * IJ], bf16, isOutput=False)  # dots layout
    wih = dp("wih", [PC_N, 128, 768], bf16, isOutput=False)
    whh = dp("whh", [HC_N, 128, 768], bf16, isOutput=False)
    wk = dp("wk", [K, HC_N, 128, P], bf16, isOutput=False)
    brz = dp("brz", [128, 4], f32, isOutput=False)
    bihn = dp("bihn", [128, 2], f32, isOutput=False)
    bhhn = dp("bhhn", [128, 2], f32, isOutput=False)
    wklo = dp("wklo", [128, K * PC_N], f32, isOutput=False)
    wkhi = dp("wkhi", [128, K * PC_N], f32, isOutput=False)
    corr = dp("corr", [70, 2 * BS * IJ], bf16, isOutput=False)
    cnt1 = dp("cnt1", [70, 2 * BS * IJ], bf16, isOutput=False)
    posm = dp("posm", [70, 2 * IJ], f32, isOutput=False)
    out = dp("out", [1, 2], f32, isOutput=True)

    with TileContext(nc, pool_alloc_mode="queue") as tc:
        with (
            tc.tile_pool(name="pers", bufs=1) as pers,
            tc.tile_pool(name="psGH", bufs=3, space="PSUM") as psGH,
        ):
            # ---- persistent small loads ----
            brz_t = pers.tile([128, 4], f32)
            nc.sync.dma_start(out=brz_t, in_=brz[:, :])
            bihn_t = pers.tile([128, 2], f32)
            nc.sync.dma_start(out=bihn_t, in_=bihn[:, :])
            bhhn_t = pers.tile([128, 2], f32)
            nc.sync.dma_start(out=bhhn_t, in_=bhhn[:, :])
            wklo_t = pers.tile([128, K * PC_N], f32)
            nc.sync.dma_start(out=wklo_t, in_=wklo[:, :])
            wkhi_t = pers.tile([128, K * PC_N], f32)
            nc.sync.dma_start(out=wkhi_t, in_=wkhi[:, :])
            whh_t = [pers.tile([128, 768], bf16, tag=f"whh{h}", name=f"whh{h}") for h in range(HC_N)]
            for h in range(HC_N):
                nc.sync.dma_start(out=whh_t[h], in_=whh[h, :, :])

            # zero initial hidden state (bf16)
            zb = pers.tile([128, 256], bf16)
            nc.vector.memset(zb, 0.0)

            # GRU context: per-(h-chunk, r-pair) tiles [128, 512] bf16;
            # each r block is 256 cols = 224 real + 32 pad (zeroed)
            ctxp = [
                [pers.tile([128, 512], bf16, tag=f"ctx{h}_{rp}", name=f"ctx{h}_{rp}") for rp in range(R // 2)]
                for h in range(HC_N)
            ]
            for h in range(HC_N):
                for rp in range(R // 2):
                    pv = ctxp[h][rp].rearrange("p (q x) -> p q x", q=2)[:, :, BC:]
                    nc.vector.memset(pv, 0.0)

            def ctx_r(h, r):
                return ctxp[h][r // 2][:, (r % 2) * 256 : (r % 2) * 256 + 256]

            outS = pers.tile([1, 2], f32)
            # gi chunks of 3 steps each (672 cols)
            GI_CH = [(0, 672), (672, 672)]
            gis = [
                [pers.tile([128, w], f32, tag=f"gis{m}_{c}", name=f"gis{m}_{c}") for c, (o, w) in enumerate(GI_CH)]
                for m in range(6)
            ]

            def gi_slice(m, r):
                ci, rem = divmod(r, 3)
                return gis[m][ci][:, rem * BC : (rem + 1) * BC]

            # ---- phase 1: gi = x @ W_ih.T, interleaved with GRU steps ----
            with (
                tc.tile_pool(name="p1", bufs=1) as p1,
                tc.tile_pool(name="psGI", bufs=2, space="PSUM") as psGI,
            ):
                enc_t = [p1.tile([128, R * BC], bf16, tag=f"enc{i}", name=f"enc{i}") for i in range(PC_N)]
                wih_t = [p1.tile([128, 768], bf16, tag=f"wih{i}", name=f"wih{i}") for i in range(PC_N)]
                for i in range(PC_N):
                    nc.sync.dma_start(out=enc_t[i], in_=encT[i, :, :])
                    nc.sync.dma_start(out=wih_t[i], in_=wih[i, :, :])

                def emit_gi_chunk(ci):
                    off, w = GI_CH[ci]
                    for m in range(6):
                        ps = psGI.tile([128, 512], f32, tag="gi", name=f"gi_{ci}_{m}")
                        for h2 in range(2):          # 672 = 336+336 (<=512 psum)
                            lo, wd = h2 * 336, 336
                            for pc in range(PC_N):
                                nc.tensor.matmul(
                                    ps[:, :wd],
                                    wih_t[pc][:, m * 128 : (m + 1) * 128],
                                    enc_t[pc][:, off + lo : off + lo + wd],
                                    start=(pc == 0),
                                    stop=(pc == PC_N - 1),
                                )
                            nc.vector.tensor_copy(
                                gis[m][ci][:, lo : lo + wd], ps[:, :wd]
                            )

                def emit_gru_step(r):
                    hprev = [zb, zb] if r == 0 else [ctx_r(h, r - 1) for h in range(HC_N)]
                    ghp = []
                    for m in range(6):
                        ps = psGH.tile([128, 256], f32, tag="gh", name=f"gh_{r}_{m}")
                        for hc in range(HC_N):
                            nc.tensor.matmul(
                                ps,
                                whh_t[hc][:, m * 128 : (m + 1) * 128],
                                hprev[hc],
                                start=(hc == 0),
                                stop=(hc == HC_N - 1),
                            )
                        ghp.append(ps)
                    for t in range(2):
                        iR = gi_slice(0 + t, r)
                        iZ = gi_slice(2 + t, r)
                        iN = gi_slice(4 + t, r)
                        hR = ghp[0 + t][:, :BC]
                        hZ = ghp[2 + t][:, :BC]
                        hN = ghp[4 + t][:, :BC]
                        tA = pers.tile([128, BC], f32, tag="tA", bufs=2, name=f"tA{r}{t}")
                        nc.vector.tensor_tensor(tA, iR, hR, op=Alu.add)
                        rt = pers.tile([128, BC], f32, tag="rt", bufs=2, name=f"rt{r}{t}")
                        nc.scalar.activation(rt, tA, Act.Sigmoid, bias=brz_t[:, 0 + t : 1 + t])
                        tB = pers.tile([128, BC], f32, tag="tB", bufs=2, name=f"tB{r}{t}")
                        nc.vector.tensor_tensor(tB, iZ, hZ, op=Alu.add)
                        zt = pers.tile([128, BC], f32, tag="zt", bufs=2, name=f"zt{r}{t}")
                        nc.scalar.activation(zt, tB, Act.Sigmoid, bias=brz_t[:, 2 + t : 3 + t])
                        tV = pers.tile([128, BC], f32, tag="tV", bufs=2, name=f"tV{r}{t}")
                        nc.vector.scalar_tensor_tensor(
                            tV, hN, bhhn_t[:, t : t + 1], rt, op0=Alu.add, op1=Alu.mult
                        )
                        tW = pers.tile([128, BC], f32, tag="tW", bufs=2, name=f"tW{r}{t}")
                        nc.vector.tensor_tensor(tW, tV, iN, op=Alu.add)
                        nt = pers.tile([128, BC], f32, tag="nt", bufs=2, name=f"nt{r}{t}")
                        nc.scalar.activation(nt, tW, Act.Tanh, bias=bihn_t[:, t : t + 1])
                        tD = pers.tile([128, BC], f32, tag="tD", bufs=2, name=f"tD{r}{t}")
                        nc.vector.tensor_tensor(tD, hprev[t][:, :BC], nt, op=Alu.subtract)
                        tE = pers.tile([128, BC], f32, tag="tE", bufs=2, name=f"tE{r}{t}")
                        nc.vector.tensor_tensor(tE, zt, tD, op=Alu.mult)
                        hout = ctx_r(t, r)[:, :BC]
                        nc.vector.tensor_tensor(hout, nt, tE, op=Alu.add)

                # interleave emission so GRU overlaps the gi tail
                emit_gi_chunk(0)
                emit_gru_step(0)
                emit_gi_chunk(1)
                for r in range(1, R):
                    emit_gru_step(r)

            # ---- phase 3: preds (clip(ctx @ Wk.T + b)) + dots + loss ----
            with (
                tc.tile_pool(name="pp", bufs=1) as ppool,
                tc.tile_pool(name="psPP", bufs=2, space="PSUM") as psPP,
                tc.tile_pool(name="psDP", bufs=2, space="PSUM") as psDP,
            ):
                predsT = [
                    ppool.tile([128, BS * HALF * C], bf16, tag=f"pt{i}", name=f"pt{i}")
                    for i in range(PC_N)
                ]
                encB_t = [
                    ppool.tile([128, BS * IJ], bf16, tag=f"eb{i}", name=f"eb{i}")
                    for i in range(PC_N)
                ]
                for i in range(PC_N):
                    nc.sync.dma_start(out=encB_t[i], in_=encB[i, :, :])
                posm_t = ppool.tile([70, 2 * IJ], f32)
                nc.sync.dma_start(out=posm_t, in_=posm[:, :])
                cnt1_t = ppool.tile([70, 2 * BS * IJ], bf16)
                nc.sync.dma_start(out=cnt1_t, in_=cnt1[:, :])
                corr_t = ppool.tile([70, 2 * BS * IJ], bf16)
                nc.sync.dma_start(out=corr_t, in_=corr[:, :])
                D = ppool.tile([70, 2 * BS * IJ], f32)
                B2 = ppool.tile([70, 2 * BS * IJ], f32)
                G2 = BS  # groups per half
                mx = ppool.tile([70, 2 * G2], f32, tag="mx")
                se = ppool.tile([70, 2 * G2], f32, tag="se")
                pos = ppool.tile([70, 2 * G2], f32, tag="pos")
                lnv = ppool.tile([70, 2 * G2], f32, tag="lnv")
                corr = ppool.tile([70, 2 * G2], f32, tag="corr")
                Ssum = ppool.tile([70, 4], f32, tag="S")

                def emit_preds_pass(pass_i):
                    ppairs = PAIRS[pass_i * HALF : (pass_i + 1) * HALF]
                    runs = []
                    q = 0
                    while q < HALF:
                        k = ppairs[q][0]
                        q0 = q
                        while q < HALF and ppairs[q][0] == k:
                            q += 1
                        runs.append((k, q0, q))
                    for k, q0, q1 in runs:
                        wk_t = []
                        for hc in range(HC_N):
                            w = ppool.tile(
                                [128, P], bf16, tag=f"wk{hc}", bufs=2,
                                name=f"wk{pass_i}_{k}_{hc}",
                            )
                            nc.sync.dma_start(out=w, in_=wk[k, hc, :, :])
                            wk_t.append(w)
                        for qc in range(q0, q1, 2):
                            nq = min(2, q1 - qc)        # 2 -> N=512, 1 -> N=256
                            rs = [ppairs[qc + i][1] for i in range(nq)]
                            for m in range(PC_N):
                                ps = psPP.tile(
                                    [128, 512], f32, tag="pp", name=f"pp_{pass_i}_{qc}_{m}"
                                )
                                for hc in range(HC_N):
                                    if nq == 2:
                                        assert rs[1] == rs[0] + 1 and rs[0] % 2 == 0
                                        rhs = ctxp[hc][rs[0] // 2]
                                    else:
                                        rhs = ctx_r(hc, rs[0])
                                    nc.tensor.matmul(
                                        ps[:, : nq * 256],
                                        wk_t[hc][:, m * 128 : (m + 1) * 128],
                                        rhs,
                                        start=(hc == 0),
                                        stop=(hc == HC_N - 1),
                                    )
                                psv = ps.rearrange("p (q x) -> p q x", q=2)[
                                    :, :nq, :BC
                                ].rearrange("p q (b c) -> p q b c", b=BS)
                                dst = predsT[m].rearrange(
                                    "p (b q c) -> p q b c", b=BS, q=HALF
                                )[:, qc : qc + nq, :, :]
                                nc.vector.tensor_scalar(
                                    dst, psv,
                                    wklo_t[:, k * PC_N + m : k * PC_N + m + 1],
                                    wkhi_t[:, k * PC_N + m : k * PC_N + m + 1],
                                    Alu.max, Alu.min,
                                )

                def emit_dots_pass(pass_i):
                    for b in range(BS):
                        ps = psDP.tile([70, IJ], f32, tag="dp", name=f"dp{pass_i}_{b}")
                        for pc in range(PC_N):
                            nc.tensor.matmul(
                                ps,
                                predsT[pc][:, b * 70 : (b + 1) * 70],
                                encB_t[pc][:, b * IJ : (b + 1) * IJ],
                                start=(pc == 0),
                                stop=(pc == PC_N - 1),
                            )
                        gsl = slice(
                            (pass_i * BS + b) * IJ, (pass_i * BS + b + 1) * IJ
                        )
                        nc.vector.tensor_tensor(D[:, gsl], ps, corr_t[:, gsl], op=Alu.add)

                def emit_post_half(h):
                    # groups of this half: cols [h*G2*IJ, (h+1)*G2*IJ)
                    lo = h * G2 * IJ
                    hi = (h + 1) * G2 * IJ
                    Dh = D[:, lo:hi]
                    B2h = B2[:, lo:hi]
                    Dv = Dh.rearrange("p (g j) -> p g j", j=IJ)
                    B2v = B2h.rearrange("p (g j) -> p g j", j=IJ)
                    cnt_h = cnt1_t[:, lo:hi]
                    gsl = slice(h * G2, (h + 1) * G2)
                    mxh = mx[:, gsl]
                    seh = se[:, gsl]
                    posh = pos[:, gsl]
                    lnvh = lnv[:, gsl]
                    corrh = corr[:, gsl]
                    nc.vector.tensor_scalar(B2h, cnt_h, 0.0, -1e30, Alu.is_equal, Alu.mult)
                    nc.vector.tensor_tensor(Dh, Dh, B2h, op=Alu.add)
                    nc.vector.tensor_reduce(mxh, Dv, axis=mybir.AxisListType.X, op=Alu.max)
                    nc.vector.tensor_tensor(
                        B2v, Dv, mxh.unsqueeze(2).broadcast_to([70, G2, IJ]), op=Alu.subtract
                    )
                    nc.scalar.activation(B2h, B2h, Act.Exp)
                    nc.vector.tensor_tensor(B2h, B2h, cnt_h, op=Alu.mult)
                    nc.vector.tensor_reduce(seh, B2v, axis=mybir.AxisListType.X, op=Alu.add)
                    # pos = sum(D * posmask) (exact: zeros elsewhere)
                    pmh = posm_t[:, h * IJ : (h + 1) * IJ]
                    nc.vector.tensor_tensor(
                        B2v, Dv, pmh.unsqueeze(1).broadcast_to([70, G2, IJ]), op=Alu.mult
                    )
                    nc.vector.tensor_reduce(posh, B2v, axis=mybir.AxisListType.X, op=Alu.add)
                    # loss = ln(se) + mx - pos ; correct = (pos >= mx)
                    nc.scalar.activation(lnvh, seh, Act.Ln)
                    nc.vector.tensor_tensor(lnvh, lnvh, mxh, op=Alu.add)
                    nc.vector.tensor_tensor(corrh, posh, mxh, op=Alu.is_ge)
                    nc.vector.tensor_tensor(lnvh, lnvh, posh, op=Alu.subtract)
                    nc.vector.tensor_reduce(
                        Ssum[:, 2 * h : 2 * h + 1], lnvh,
                        axis=mybir.AxisListType.X, op=Alu.add,
                    )
                    nc.vector.tensor_reduce(
                        Ssum[:, 2 * h + 1 : 2 * h + 2], corrh,
                        axis=mybir.AxisListType.X, op=Alu.add,
                    )

                emit_preds_pass(0)
                emit_dots_pass(0)
                emit_preds_pass(1)
                emit_post_half(0)
                emit_dots_pass(1)
                emit_post_half(1)

                # combine halves: [loss, acc] = colsums of Ssum (half0|half1)
                ones = ppool.tile([70, 1], f32, tag="ones")
                nc.vector.memset(ones, 1.0)
                fp = psDP.tile([1, 4], f32, tag="dp", name="fin")
                nc.tensor.matmul(fp, ones, Ssum, start=True, stop=True)
                fs = ppool.tile([1, 4], f32, tag="fs")
                nc.vector.tensor_copy(fs, fp)
                nc.vector.tensor_tensor(outS, fs[:, 0:2], fs[:, 2:4], op=Alu.add)
                nc.sync.dma_start(out=out[:, :], in_=outS)

    nc.finalize()
    return nc


def _prep_inputs(encodings, hidden, W_ih, W_hh, b_ih, b_hh, Wk_w, Wk_b,
                 neg_rows, neg_cols):
    """Host-side reformat of the full inputs into per-core DMA-clean arrays."""
    bf16 = np.float16
    enc = np.ascontiguousarray(encodings, dtype=np.float32)
    e6 = enc.reshape(NCORE, BS, C, C, PC_N, 128)  # (core, b, i, c, pc, pp)
    # GRU layout: [core, pc, pp, r*BC + b*7 + c], r < 6
    encT = np.ascontiguousarray(
        e6[:, :, :R].transpose(0, 4, 5, 2, 1, 3)
    ).reshape(NCORE, PC_N, 128, R * BC).astype(bf16)
    # dots layout: [core, pc, pp, b*49 + i*7 + c]
    encB = np.ascontiguousarray(
        e6.transpose(0, 4, 5, 1, 2, 3)
    ).reshape(NCORE, PC_N, 128, BS * IJ).astype(bf16)

    wih = np.ascontiguousarray(W_ih.T, dtype=np.float32).reshape(PC_N, 128, 768).astype(bf16)
    whh = np.ascontiguousarray(W_hh.T, dtype=np.float32).reshape(HC_N, 128, 768).astype(bf16)
    wkh = np.ascontiguousarray(
        Wk_w.transpose(0, 2, 1), dtype=np.float32
    ).reshape(K, HC_N, 128, P).astype(bf16)
    bsum = (b_ih + b_hh).astype(np.float32)
    brz = np.ascontiguousarray(bsum[:512].reshape(4, 128).T)
    bihn = np.ascontiguousarray(b_ih[512:].astype(np.float32).reshape(2, 128).T)
    bhhn = np.ascontiguousarray(b_hh[512:].astype(np.float32).reshape(2, 128).T)
    wkbT = np.ascontiguousarray(
        Wk_b.astype(np.float32).reshape(K, PC_N, 128).transpose(2, 0, 1)
    ).reshape(128, K * PC_N)
    wklo = -1.0 - wkbT
    wkhi = 1.0 - wkbT
    # rank-1 bias correction: corr[k, b, ij] = sum_p Wk_b[k,p] * enc[b,i,j,p]
    corr_k = np.einsum(
        "kp,bijp->kbij", Wk_b.astype(np.float32), enc, optimize=True
    ).reshape(K, B, IJ)
    # expand to device layout [core, row=q*7+c, half, b_local, j] (k by pair)
    corr_dev = np.empty((NCORE, HALF * C, 2, BS, IJ), dtype=np.float32)
    for half in range(2):
        for qq in range(HALF):
            k, _r = PAIRS[half * HALF + qq]
            for c in range(C):
                corr_dev[:, qq * 7 + c, half] = corr_k[k].reshape(NCORE, BS, IJ)
    corr_dev = corr_dev.reshape(NCORE, HALF * C, 2 * BS * IJ).astype(bf16)

    # negatives -> multiplicity counts over the 49 cells, plus the positive
    neg_idx = (neg_rows.astype(np.int64) * 7 + neg_cols.astype(np.int64))  # [B,K,R,C,63]
    sel = np.stack([neg_idx[:, k, r] for (k, r) in PAIRS], axis=1)  # [B,20,C,63]
    flat = (
        np.arange(B * NPAIR * C, dtype=np.int64)[:, None] * IJ
        + sel.reshape(B * NPAIR * C, S - 1)
    ).ravel()
    cnts = np.bincount(flat, minlength=B * NPAIR * C * IJ).reshape(
        B, NPAIR, C, IJ
    ).astype(np.float32)
    cvec = np.arange(C)
    for pi, (k, r) in enumerate(PAIRS):
        cnts[:, pi, cvec, r * 7 + cvec] += 1.0   # include the positive
    # device layout [core, row=q*7+c, half, b_local, j]
    cnt1 = np.ascontiguousarray(
        cnts.reshape(NCORE, BS, 2, HALF, C, IJ).transpose(0, 3, 4, 2, 1, 5)
    ).reshape(NCORE, HALF * C, 2 * BS * IJ).astype(bf16)

    posm = np.zeros((HALF * C, 2, IJ), dtype=np.float32)
    for half in range(2):
        for qq in range(HALF):
            k, r = PAIRS[half * HALF + qq]
            for c in range(C):
                posm[qq * 7 + c, half, r * 7 + c] = 1.0
    posm = posm.reshape(HALF * C, 2 * IJ)

    in_maps = []
    for core in range(NCORE):
        in_maps.append(
            {
                "encT": encT[core],
                "encB": encB[core],
                "wih": wih,
                "whh": whh,
                "wk": wkh,
                "brz": brz,
                "bihn": bihn,
                "bhhn": bhhn,
                "wklo": wklo,
                "wkhi": wkhi,
                "corr": corr_dev[core],
                "cnt1": cnt1[core],
                "posm": posm,
            }
        )
    return in_maps


def _get_program():
    if "nc" not in _CACHE:
        _CACHE["nc"] = _build_program()
    return _CACHE["nc"]


def run_on_device(in_maps, trace=False, tmpdir=None):
    from concourse.bass_utils import run_bass_kernel_spmd

    nc = _get_program()
    return run_bass_kernel_spmd(
        nc, in_maps, list(range(NCORE)), trace=trace, tmpdir=tmpdir
    )


def kernel(**inputs):
    in_maps = _prep_inputs(**inputs)
    res = run_on_device(in_maps)
    loss_sum = 0.0
    corr_sum = 0.0
    for core in range(NCORE):
        o = res.results[core]["out"]
        loss_sum += float(o[0, 0])
        corr_sum += float(o[0, 1])
    loss = np.float32(loss_sum / N_PREDS)
    acc = np.float32(corr_sum / N_PREDS)
    return loss, acc
